# revision 9
# baseline (speedup 1.0000x reference)
"""Boundary-map kernel for Trainium2 (Bass/Tile), 8-core SPMD.

Math: a pixel is an edge pixel iff some 4-adjacent pair of pixels with
different labels lies inside its radius-2 Euclidean disk (on the 2-zero-padded
label map; verified exhaustively against the reference).  With
    XH(i,j) = x(i,j) ^ x(i,j+1)      (horizontal pair diffs)
    XV(i,j) = x(i,j) ^ x(i+1,j)      (vertical pair diffs)
    a = XH(0,-1) | XH(0,0)           c = XV(-1,0) | XV(0,0)
    m = a | c
    edge = OR_{s in PLUS} m(p+s) != 0,   PLUS = {(0,+-1),(+-1,0)}
(the two pair-dilation sets SH/SV both factor through the same plus-shaped
dilation of m — 8 tensor_tensor ops total).

Layout: rows in partitions; columns byte-PACKED 2-per-int16 lane (lo byte =
left half, hi byte = right half of the image, each half carrying its own
2-col halo), so every DVE op processes 256 px per free element at the 2x
16-bit mode.  All values stay bytes (labels < 32, ORs of XORs < 32); the
output bytes are nonzero-iff-edge and the host binarizes during assembly.
Vertical (cross-partition) shifts are done with HBM re-loads offset by one
row (x+) and SBUF->SBUF partition-shifted DMA copies (XVm, mm, mp) since
engine access patterns cannot start at partition offsets != 0 mod 32.

Sharding: 2 batches x 4 col-quarters -> 8 cores.  Each core: two 125-out-row
bands (full width, tiles [128p, 1028f]) + one 24-row x 512-col strip
([28p, 260f]) covering the last 24 rows of its batch.
"""

import numpy as np
from contextlib import ExitStack

import concourse.bass as bass
import concourse.bacc as bacc
import concourse.mybir as mybir
import concourse.tile as tile
from concourse import bass_utils

I16 = mybir.dt.int16
I32 = mybir.dt.int32
OP = mybir.AluOpType

B, H, W = 2, 1024, 2048
NCORES = 8
BAND = 125           # output rows per main band
NBAND = 8            # bands per batch
F = 1028             # packed free width for main bands (2 planes of 1028 cols)
SROWS = H - NBAND * BAND   # 24 strip rows per batch
SF = 260             # strip packed free width (2 planes of 260)

PROFILE = False
LAST_EXEC_NS = None
LAST_RESULTS = None


def _band_job(nc, sb, src, P, PV, C, dst):
    """One band: src packed int16 in HBM (row i = padded-array row r0+i).
    Computes edge bytes for tile partitions 2..PV-1, cols 2..C-3, DMAs them
    to dst.  PV = valid partition extent for the vertical chain (main bands:
    128 with a 129-row src; strip: 27 with a 28-row src)."""
    x = sb.tile([P, C], I16, tag="x")
    xp = sb.tile([P, C], I16, tag="xp")
    nc.sync.dma_start(x[:, :], src[0:P, :])
    nc.scalar.dma_start(xp[0:PV, :], src[1:PV + 1, :])

    XH = sb.tile([P, C], I16, tag="xh")
    nc.vector.tensor_tensor(out=XH[:, 0:C - 1], in0=x[:, 0:C - 1],
                            in1=x[:, 1:C], op=OP.bitwise_xor)
    XV = sb.tile([P, C], I16, tag="xv")
    nc.vector.tensor_tensor(out=XV[0:PV, :], in0=x[0:PV, :],
                            in1=xp[0:PV, :], op=OP.bitwise_xor)

    XVm = sb.tile([P, C], I16, tag="xvm")
    nc.sync.dma_start(XVm[1:PV, :], XV[0:PV - 1, :])
    nc.scalar.dma_start(XVm[0:1, :], XV[0:1, :])

    a = sb.tile([P, C], I16, tag="a")
    nc.vector.memset(a[:, 0:1], 0)
    nc.vector.memset(a[:, C - 1:C], 0)
    nc.vector.tensor_tensor(out=a[:, 1:C - 1], in0=XH[:, 0:C - 2],
                            in1=XH[:, 1:C - 1], op=OP.bitwise_or)
    # Pool has no bitwise/int16 ops; run c and m as int32 adds on bitcast
    # views (values are small nonneg bytes, sums stay < 256 per byte so
    # packed lanes never carry: XH,XV<32 -> a,c<64 -> m<128 -> q,r<256;
    # the final combine stays a DVE bitwise_or).
    cc = sb.tile([P, C], I16, tag="cc")
    nc.gpsimd.tensor_tensor(out=cc[0:PV, :].bitcast(I32),
                            in0=XVm[0:PV, :].bitcast(I32),
                            in1=XV[0:PV, :].bitcast(I32), op=OP.add)
    m = sb.tile([P, C], I16, tag="m")
    nc.gpsimd.tensor_tensor(out=m[0:PV, :].bitcast(I32),
                            in0=a[0:PV, :].bitcast(I32),
                            in1=cc[0:PV, :].bitcast(I32), op=OP.add)

    q = sb.tile([P, C], I16, tag="q")
    nc.vector.tensor_tensor(out=q[0:PV, 2:C - 2], in0=m[0:PV, 1:C - 3],
                            in1=m[0:PV, 3:C - 1], op=OP.bitwise_or)

    mm = sb.tile([P, C], I16, tag="mm")
    mp = sb.tile([P, C], I16, tag="mp")
    nc.sync.dma_start(mm[1:PV, :], m[0:PV - 1, :])
    nc.sync.dma_start(mm[0:1, :], m[0:1, :])
    nc.scalar.dma_start(mp[0:PV - 1, :], m[1:PV, :])
    nc.scalar.dma_start(mp[PV - 1:PV, :], m[PV - 1:PV, :])

    r = sb.tile([P, C], I16, tag="r")
    nc.vector.tensor_tensor(out=r[0:PV, 2:C - 2], in0=mm[0:PV, 2:C - 2],
                            in1=mp[0:PV, 2:C - 2], op=OP.bitwise_or)
    e = sb.tile([P, C], I16, tag="e")
    nc.vector.tensor_tensor(out=e[0:PV, 2:C - 2], in0=q[0:PV, 2:C - 2],
                            in1=r[0:PV, 2:C - 2], op=OP.bitwise_or)

    nc.sync.dma_start(dst, e[2:PV - 1, 2:C - 2])


def build_nc():
    # Bacc: its compile() legalizes multi-wait instructions via
    # generate_event_semaphores (the TileContext tail drain needs it).
    nc = bacc.Bacc("TRN2", target_bir_lowering=False, debug=False)
    s0 = nc.dram_tensor("s0", [BAND + 4, F], I16, kind="ExternalInput").ap()
    s1 = nc.dram_tensor("s1", [BAND + 4, F], I16, kind="ExternalInput").ap()
    ss = nc.dram_tensor("ss", [SROWS + 4, SF], I16, kind="ExternalInput").ap()
    y0 = nc.dram_tensor("y0", [BAND, F - 4], I16, kind="ExternalOutput").ap()
    y1 = nc.dram_tensor("y1", [BAND, F - 4], I16, kind="ExternalOutput").ap()
    ys = nc.dram_tensor("ys", [SROWS, SF - 4], I16, kind="ExternalOutput").ap()

    with ExitStack() as ctx:
        tc = ctx.enter_context(tile.TileContext(nc))
        sb = ctx.enter_context(tc.tile_pool(name="sb", bufs=2))
        _band_job(nc, sb, s0, 128, 128, F, y0)
        _band_job(nc, sb, s1, 128, 128, F, y1)
        _band_job(nc, sb, ss, SROWS + 4, SROWS + 3, SF, ys)
    nc.compile()
    return nc


def make_in_maps(gtmasks):
    lab = np.asarray(gtmasks)[:, 0].astype(np.uint8)  # labels 0..19 fit a byte
    packed = []
    strips = []
    for b in range(B):
        A = np.pad(lab[b], 2)  # [H+4, W+4] = [1028, 2052]
        P = (A[:, 0:F].astype(np.uint16)
             | (A[:, W // 2:W // 2 + F].astype(np.uint16) << 8)).view(np.int16)
        packed.append(P)
        strips.append(A)
    in_maps = []
    for c in range(NCORES):
        b, qq = divmod(c, 4)
        A = strips[b]
        r0 = NBAND * BAND  # first strip out-row (padded-array row r0+... )
        c0 = (W // NCORES * 2) * qq  # 512*qq
        slo = A[r0:r0 + SROWS + 4, c0:c0 + SF]
        shi = A[r0:r0 + SROWS + 4, c0 + SF - 4:c0 + 2 * SF - 4]
        sp = (slo.astype(np.uint16) | (shi.astype(np.uint16) << 8)).view(np.int16)
        im = {
            "s0": np.ascontiguousarray(packed[b][BAND * 2 * qq:BAND * 2 * qq + BAND + 4, :]),
            "s1": np.ascontiguousarray(packed[b][BAND * (2 * qq + 1):BAND * (2 * qq + 1) + BAND + 4, :]),
            "ss": np.ascontiguousarray(sp),
        }
        in_maps.append(im)
    return in_maps


def assemble(results):
    out = np.zeros((B, 1, H, W), np.int32)
    for c in range(NCORES):
        b, qq = divmod(c, 4)
        for j, k in enumerate((2 * qq, 2 * qq + 1)):
            v = results[c][f"y{j}"].view(np.uint8).reshape(BAND, F - 4, 2)
            rows = slice(BAND * k, BAND * (k + 1))
            out[b, 0, rows, 0:W // 2] = v[:, :, 0] != 0
            out[b, 0, rows, W // 2:W] = v[:, :, 1] != 0
        vs = results[c]["ys"].view(np.uint8).reshape(SROWS, SF - 4, 2)
        c0 = 512 * qq
        out[b, 0, NBAND * BAND:H, c0:c0 + 256] = vs[:, :, 0] != 0
        out[b, 0, NBAND * BAND:H, c0 + 256:c0 + 512] = vs[:, :, 1] != 0
    return out


def kernel(gtmasks):
    global LAST_EXEC_NS, LAST_RESULTS
    in_maps = make_in_maps(gtmasks)
    nc = build_nc()
    res = bass_utils.run_bass_kernel_spmd(
        nc, in_maps, core_ids=list(range(NCORES)), trace=PROFILE)
    LAST_EXEC_NS = res.exec_time_ns
    LAST_RESULTS = res
    return assemble(res.results)


# revision 12
# speedup vs baseline: 1.1432x; 1.1432x over previous
"""Boundary-map kernel for Trainium2 (Bass/Tile), 8-core SPMD.

Math: a pixel is an edge pixel iff some 4-adjacent pair of pixels with
different labels lies inside its radius-2 Euclidean disk (on the 2-zero-padded
label map; verified exhaustively against the reference).  With
    XH(i,j) = x(i,j) ^ x(i,j+1)      (horizontal pair diffs)
    XV(i,j) = x(i,j) ^ x(i+1,j)      (vertical pair diffs)
    a = XH(0,-1) | XH(0,0)           c = XV(-1,0) | XV(0,0)
    m = a | c
    edge = OR_{s in PLUS} m(p+s) != 0,   PLUS = {(0,+-1),(+-1,0)}
(the two pair-dilation sets SH/SV both factor through the same plus-shaped
dilation of m — 8 tensor_tensor ops total).

Layout: rows in partitions; columns byte-PACKED 2-per-int16 lane (lo byte =
left half, hi byte = right half of the image, each half carrying its own
2-col halo), so every DVE op processes 256 px per free element at the 2x
16-bit mode.  All values stay bytes (labels < 32, ORs/sums of XORs < 256);
the output bytes are nonzero-iff-edge and the host binarizes during assembly.
Since engine access patterns cannot start at partition offsets != 0 mod 32,
vertical (cross-partition) shifts come from: three row-shifted HBM loads of
the same slab (x-, x, x+ -> XV and XVb = XV shifted down, merged into c with
no dependent copy), plus SBUF->SBUF partition-shifted DMA copies of m (mm,
mp) for the final vertical dilation.  Pool has no bitwise/int16 support, so
the ops it takes (c, m) run as int32 adds on bitcast views (carry-free by
the byte bounds above).  Ops are emitted stage-major across the three jobs
so each engine's queue never blocks a later-ready transfer behind an
unsatisfied wait.

Sharding: 2 batches x 4 col-quarters -> 8 cores.  Each core: two 125-out-row
bands (full width, tiles [128p, 1028f]) + one 24-row x 512-col strip
([28p, 260f]) covering the last 24 rows of its batch.
"""

import numpy as np
from contextlib import ExitStack

import concourse.bass as bass
import concourse.bacc as bacc
import concourse.mybir as mybir
import concourse.tile as tile
from concourse import bass_utils

I16 = mybir.dt.int16
I32 = mybir.dt.int32
OP = mybir.AluOpType

B, H, W = 2, 1024, 2048
NCORES = 8
BAND = 125           # output rows per main band
NBAND = 8            # bands per batch
F = 1028             # packed free width for main bands (2 planes of 1028 cols)
SROWS = H - NBAND * BAND   # 24 strip rows per batch
SF = 260             # strip packed free width (2 planes of 260)

PROFILE = False
LAST_EXEC_NS = None
LAST_RESULTS = None


class Job:
    """State for one band job.  src rows: 0 = padded-array row r0-1 (x-),
    x = src[1:P+1], x+ = src[2:PV+2].  Tile partition p = padded row r0+p."""

    def __init__(self, sb, src, P, PV, C, dst, tag):
        self.src, self.P, self.PV, self.C, self.dst = src, P, PV, C, dst
        for name in ("x", "xm", "xp", "XH", "XV", "XVb", "a", "cc", "m",
                     "q", "mm", "mp", "r", "e"):
            setattr(self, name, sb.tile([P, C], I16, name=f"{name}{tag}",
                                        tag=f"{name}{tag}"))


def build_nc():
    # Bacc: its compile() legalizes multi-wait instructions via
    # generate_event_semaphores (the TileContext tail drain needs it).
    nc = bacc.Bacc("TRN2", target_bir_lowering=False, debug=False)
    s0 = nc.dram_tensor("s0", [BAND + 5, F], I16, kind="ExternalInput").ap()
    s1 = nc.dram_tensor("s1", [BAND + 5, F], I16, kind="ExternalInput").ap()
    ss = nc.dram_tensor("ss", [SROWS + 5, SF], I16, kind="ExternalInput").ap()
    y0 = nc.dram_tensor("y0", [BAND, F - 4], I16, kind="ExternalOutput").ap()
    y1 = nc.dram_tensor("y1", [BAND, F - 4], I16, kind="ExternalOutput").ap()
    ys = nc.dram_tensor("ys", [SROWS, SF - 4], I16, kind="ExternalOutput").ap()

    with ExitStack() as ctx:
        tc = ctx.enter_context(tile.TileContext(nc))
        sb = ctx.enter_context(tc.tile_pool(name="sb", bufs=1))
        jobs = [
            Job(sb, s0, 128, 128, F, y0, "0"),
            Job(sb, s1, 128, 128, F, y1, "1"),
            Job(sb, ss, SROWS + 4, SROWS + 3, SF, ys, "s"),
        ]
        # stage 0: all loads (independent; x/xm on Sync, xp on Scalar)
        for j in jobs:
            nc.sync.dma_start(j.x[:, :], j.src[1:j.P + 1, :])
            nc.sync.dma_start(j.xm[:, :], j.src[0:j.P, :])
            nc.scalar.dma_start(j.xp[0:j.PV, :], j.src[2:j.PV + 2, :])
        # stage 1: diffs (DVE)
        for j in jobs:
            P, PV, C = j.P, j.PV, j.C
            nc.vector.tensor_tensor(out=j.XH[:, 0:C - 1], in0=j.x[:, 0:C - 1],
                                    in1=j.x[:, 1:C], op=OP.bitwise_xor)
            nc.vector.tensor_tensor(out=j.XV[0:PV, :], in0=j.x[0:PV, :],
                                    in1=j.xp[0:PV, :], op=OP.bitwise_xor)
            nc.vector.tensor_tensor(out=j.XVb[0:PV, :], in0=j.xm[0:PV, :],
                                    in1=j.x[0:PV, :], op=OP.bitwise_xor)
        # stage 2: a (DVE, with edge-lane init for the int32-view consumer)
        for j in jobs:
            P, C = j.P, j.C
            nc.vector.memset(j.a[:, 0:1], 0)
            nc.vector.memset(j.a[:, C - 1:C], 0)
            nc.vector.tensor_tensor(out=j.a[:, 1:C - 1], in0=j.XH[:, 0:C - 2],
                                    in1=j.XH[:, 1:C - 1], op=OP.bitwise_or)
        # stage 3: c then m (Pool int32 adds)
        for j in jobs:
            PV = j.PV
            nc.gpsimd.tensor_tensor(out=j.cc[0:PV, :].bitcast(I32),
                                    in0=j.XVb[0:PV, :].bitcast(I32),
                                    in1=j.XV[0:PV, :].bitcast(I32), op=OP.add)
        for j in jobs:
            PV = j.PV
            nc.gpsimd.tensor_tensor(out=j.m[0:PV, :].bitcast(I32),
                                    in0=j.a[0:PV, :].bitcast(I32),
                                    in1=j.cc[0:PV, :].bitcast(I32), op=OP.add)
        # stage 4: partition-shifted copies of m (DMA) + q (DVE, off-path)
        for j in jobs:
            PV = j.PV
            nc.sync.dma_start(j.mm[1:PV, :], j.m[0:PV - 1, :])
            nc.scalar.dma_start(j.mp[0:PV - 1, :], j.m[1:PV, :])
            nc.vector.tensor_tensor(out=j.q[0:PV, 2:j.C - 2],
                                    in0=j.m[0:PV, 1:j.C - 3],
                                    in1=j.m[0:PV, 3:j.C - 1], op=OP.bitwise_or)
        # stage 5: r, e (DVE), out
        for j in jobs:
            PV, C = j.PV, j.C
            nc.vector.tensor_tensor(out=j.r[0:PV, 2:C - 2], in0=j.mm[0:PV, 2:C - 2],
                                    in1=j.mp[0:PV, 2:C - 2], op=OP.bitwise_or)
            nc.vector.tensor_tensor(out=j.e[0:PV, 2:C - 2], in0=j.q[0:PV, 2:C - 2],
                                    in1=j.r[0:PV, 2:C - 2], op=OP.bitwise_or)
            nc.scalar.dma_start(j.dst, j.e[2:PV - 1, 2:C - 2])
    nc.compile()
    return nc


def make_in_maps(gtmasks):
    lab = np.asarray(gtmasks)[:, 0].astype(np.uint8)  # labels 0..19 fit a byte
    packed = []
    raw = []
    for b in range(B):
        A = np.pad(lab[b], 2)  # [H+4, W+4] = [1028, 2052]
        # row -1 clamp ring so band 0's x- load stays in range (junk rows
        # only feed non-output partitions)
        A1 = np.vstack([A[0:1], A])  # [1029, 2052]; row i = padded row i-1
        P = (A1[:, 0:F].astype(np.uint16)
             | (A1[:, W // 2:W // 2 + F].astype(np.uint16) << 8)).view(np.int16)
        packed.append(P)
        raw.append(A1)
    in_maps = []
    for c in range(NCORES):
        b, qq = divmod(c, 4)
        A1 = raw[b]
        r0 = NBAND * BAND  # first strip out-row
        c0 = 512 * qq
        slo = A1[r0:r0 + SROWS + 5, c0:c0 + SF]
        shi = A1[r0:r0 + SROWS + 5, c0 + SF - 4:c0 + 2 * SF - 4]
        sp = (slo.astype(np.uint16) | (shi.astype(np.uint16) << 8)).view(np.int16)
        k0, k1 = 2 * qq, 2 * qq + 1
        im = {
            "s0": np.ascontiguousarray(packed[b][BAND * k0:BAND * k0 + BAND + 5, :]),
            "s1": np.ascontiguousarray(packed[b][BAND * k1:BAND * k1 + BAND + 5, :]),
            "ss": np.ascontiguousarray(sp),
        }
        in_maps.append(im)
    return in_maps


def assemble(results):
    out = np.zeros((B, 1, H, W), np.int32)
    for c in range(NCORES):
        b, qq = divmod(c, 4)
        for j, k in enumerate((2 * qq, 2 * qq + 1)):
            v = results[c][f"y{j}"].view(np.uint8).reshape(BAND, F - 4, 2)
            rows = slice(BAND * k, BAND * (k + 1))
            out[b, 0, rows, 0:W // 2] = v[:, :, 0] != 0
            out[b, 0, rows, W // 2:W] = v[:, :, 1] != 0
        vs = results[c]["ys"].view(np.uint8).reshape(SROWS, SF - 4, 2)
        c0 = 512 * qq
        out[b, 0, NBAND * BAND:H, c0:c0 + 256] = vs[:, :, 0] != 0
        out[b, 0, NBAND * BAND:H, c0 + 256:c0 + 512] = vs[:, :, 1] != 0
    return out


def kernel(gtmasks):
    global LAST_EXEC_NS, LAST_RESULTS
    in_maps = make_in_maps(gtmasks)
    nc = build_nc()
    res = bass_utils.run_bass_kernel_spmd(
        nc, in_maps, core_ids=list(range(NCORES)), trace=PROFILE)
    LAST_EXEC_NS = res.exec_time_ns
    LAST_RESULTS = res
    return assemble(res.results)


# revision 13
# speedup vs baseline: 2.3606x; 2.0648x over previous
"""Boundary-map kernel for Trainium2 (Bass/Tile), 8-core SPMD.

Math: a pixel is an edge pixel iff some 4-adjacent pair of pixels with
different labels lies inside its radius-2 Euclidean disk (on the 2-zero-padded
label map; verified exhaustively against the reference).  With
    XH(i,j) = x(i,j) ^ x(i,j+1)      (horizontal pair diffs)
    XV(i,j) = x(i,j) ^ x(i+1,j)      (vertical pair diffs)
    edge(p) = [ sum_{s in SH} XH(p+s) + sum_{s in SV} XV(p+s) ] > 0
    SH = {(0,-2),(0,-1),(0,0),(0,1),(+-1,-1),(+-1,0)}
    SV = {(-2,0),(-1,0),(0,0),(1,0),(-1,+-1),(0,+-1)}

Layout: rows in partitions; columns byte-PACKED 2-per-int16 lane (lo byte =
left image half, hi byte = right half, each half with its own 2-col halo).
DVE computes XH/XV (packed int16 xor, 2x mode) plus two fused column-pair
sums u = XH(0,-1)+XH(0,0) and s = XV(0,-1)+XV(0,+1) (carry-free: bytes<64).
All row mixing runs on the TensorEngine as fp8 band matmuls over the int8
byte view reinterpreted as float8e4: every byte is < 128 so it reads as a
NONNEGATIVE fp8 value that is zero iff the byte is zero; sums of such values
are positive iff any contributing byte is nonzero, which is all the > 0 test
needs (verified on HW incl. the denormal byte range 1..7).  Per 512-byte
PSUM chunk, 5 accumulating matmuls cover both dilation sets:
    w_v3.u8[b]  ->  SH terms (0,-1),(0,0),(+-1,-1),(+-1,0)
    I.XH8[b-4]  ->  SH (0,-2)        I.XH8[b+2]  ->  SH (0,+1)
    w_v4.XV8[b] ->  SV (-2..1, 0)    w_v2.s8[b]  ->  SV (-1,+-1),(0,+-1)
ScalarE extracts Sign(PSUM) -> int8 0/1, DMAed out (host just casts).
No SBUF->SBUF DMA anywhere (it measured ~20-30 GB/s — pathologically slow);
the only DMAs are parallel HBM loads, the weight load, and outputs.

Sharding: 2 batches x 4 col-quarters -> 8 cores.  Each core: two 125-out-row
bands (full width, tiles [128p, 1028f]) + one 24-row x 512-col strip
([28p, 260f]) covering the last 24 rows of its batch.
"""

import numpy as np
import ml_dtypes
from contextlib import ExitStack

import concourse.bass as bass
import concourse.bacc as bacc
import concourse.mybir as mybir
import concourse.tile as tile
from concourse import bass_utils

I16 = mybir.dt.int16
I8 = mybir.dt.int8
F32 = mybir.dt.float32
FP8 = mybir.dt.float8e4
OP = mybir.AluOpType
AF = mybir.ActivationFunctionType

B, H, W = 2, 1024, 2048
NCORES = 8
BAND = 125           # output rows per main band
NBAND = 8            # bands per batch
F = 1028             # packed free width for main bands (2 planes of 1028 cols)
SROWS = H - NBAND * BAND   # 24 strip rows per batch
SF = 260             # strip packed free width (2 planes of 260)
CHUNK = 512          # PSUM bank width in fp32

PROFILE = False
LAST_EXEC_NS = None
LAST_RESULTS = None

WNAMES = ("w_v3", "w_v4", "w_v2", "w_i")


def _band(taps, P=128):
    w = np.zeros((P, P), np.float32)  # [k, m]: out row m sums w[k,m]*src[k]
    for m in range(P):
        for t in taps:
            k = m + t
            if 0 <= k < P:
                w[k, m] = 1.0
    return w.astype(ml_dtypes.float8_e4m3fn)


def make_weights():
    wd = {
        "w_v3": _band([-1, 0, 1]),
        "w_v4": _band([-2, -1, 0, 1]),
        "w_v2": _band([-1, 0]),
        "w_i": _band([0]),
    }
    return np.concatenate([wd[k] for k in WNAMES], axis=1)


def _job(nc, sb, ps, wt, src, P, C, dst, V):
    """One band: src [P+2, C] packed int16 in HBM; tile partition p = source
    row p+1.  Emits edge bytes for partitions 2..P-2 (V = P-3 rows), byte
    cols 4..2C-5, to dst [V, 2C-8] int8."""
    CB = 2 * C
    x = sb.tile([P, C], I16, name="x", tag="x")
    xp = sb.tile([P, C], I16, name="xp", tag="xp")
    nc.sync.dma_start(x[:, :], src[1:P + 1, :])
    nc.sync.dma_start(xp[:, :], src[2:P + 2, :])

    XH = sb.tile([P, C], I16, name="XH", tag="XH")
    nc.vector.tensor_tensor(out=XH[:, 0:C - 1], in0=x[:, 0:C - 1],
                            in1=x[:, 1:C], op=OP.bitwise_xor)
    XV = sb.tile([P, C], I16, name="XV", tag="XV")
    nc.vector.tensor_tensor(out=XV[:, :], in0=x[:, :], in1=xp[:, :],
                            op=OP.bitwise_xor)
    u = sb.tile([P, C], I16, name="u", tag="u")
    nc.vector.tensor_tensor(out=u[:, 1:C], in0=XH[:, 0:C - 1],
                            in1=XH[:, 1:C], op=OP.add)
    s = sb.tile([P, C], I16, name="s", tag="s")
    nc.vector.tensor_tensor(out=s[:, 1:C - 1], in0=XV[:, 0:C - 2],
                            in1=XV[:, 2:C], op=OP.add)

    u8 = u[:, :].bitcast(FP8)
    s8 = s[:, :].bitcast(FP8)
    XH8 = XH[:, :].bitcast(FP8)
    XV8 = XV[:, :].bitcast(FP8)

    e8 = sb.tile([P, CB], I8, name="e8", tag="e8")
    pe = ps.tile([128, 4 * CHUNK], F32, name="pe", tag="pe")
    nout = CB - 8  # valid out bytes 4 .. 2C-5
    for c0 in range(0, nout, CHUNK):
        n = min(CHUNK, nout - c0)
        b0 = 4 + c0
        pc = pe[0:P, c0:c0 + n]
        nc.tensor.matmul(out=pc, lhsT=wt["w_v3"][0:P, 0:P], rhs=u8[:, b0:b0 + n],
                         start=True, stop=False)
        nc.tensor.matmul(out=pc, lhsT=wt["w_i"][0:P, 0:P], rhs=XH8[:, b0 - 4:b0 - 4 + n],
                         start=False, stop=False)
        nc.tensor.matmul(out=pc, lhsT=wt["w_i"][0:P, 0:P], rhs=XH8[:, b0 + 2:b0 + 2 + n],
                         start=False, stop=False)
        nc.tensor.matmul(out=pc, lhsT=wt["w_v4"][0:P, 0:P], rhs=XV8[:, b0:b0 + n],
                         start=False, stop=False)
        nc.tensor.matmul(out=pc, lhsT=wt["w_v2"][0:P, 0:P], rhs=s8[:, b0:b0 + n],
                         start=False, stop=True)
        nc.scalar.activation(out=e8[:, b0:b0 + n], in_=pc, func=AF.Sign)

    nc.gpsimd.dma_start(dst, e8[2:2 + V, 4:4 + nout])


def build_nc():
    # Bacc: its compile() legalizes multi-wait instructions via
    # generate_event_semaphores (the TileContext tail drain needs it).
    nc = bacc.Bacc("TRN2", target_bir_lowering=False, debug=False)
    s0 = nc.dram_tensor("s0", [130, F], I16, kind="ExternalInput").ap()
    s1 = nc.dram_tensor("s1", [130, F], I16, kind="ExternalInput").ap()
    ss = nc.dram_tensor("ss", [SROWS + 6, SF], I16, kind="ExternalInput").ap()
    wcat = nc.dram_tensor("wcat", [128, 128 * len(WNAMES)], FP8,
                          kind="ExternalInput").ap()
    y0 = nc.dram_tensor("y0", [BAND, 2 * F - 8], I8, kind="ExternalOutput").ap()
    y1 = nc.dram_tensor("y1", [BAND, 2 * F - 8], I8, kind="ExternalOutput").ap()
    ys = nc.dram_tensor("ys", [SROWS, 2 * SF - 8], I8, kind="ExternalOutput").ap()

    with ExitStack() as ctx:
        tc = ctx.enter_context(tile.TileContext(nc))
        wp = ctx.enter_context(tc.tile_pool(name="wp", bufs=1))
        sb = ctx.enter_context(tc.tile_pool(name="sb", bufs=3))
        ps = ctx.enter_context(tc.tile_pool(name="ps", bufs=2, space="PSUM"))
        wtile = wp.tile([128, 128 * len(WNAMES)], FP8, name="wtile")
        nc.sync.dma_start(wtile[:, :], wcat)
        wt = {k: wtile[:, 128 * i:128 * (i + 1)] for i, k in enumerate(WNAMES)}
        _job(nc, sb, ps, wt, s0, 128, F, y0, BAND)
        _job(nc, sb, ps, wt, s1, 128, F, y1, BAND)
        _job(nc, sb, ps, wt, ss, SROWS + 4, SF, ys, SROWS)
    nc.compile()
    return nc


def make_in_maps(gtmasks):
    lab = np.asarray(gtmasks)[:, 0].astype(np.uint8)  # labels 0..19 fit a byte
    wcat = make_weights()
    packed = []
    raw = []
    for b in range(B):
        A = np.pad(lab[b], 2)  # [H+4, W+4] = [1028, 2052]
        # clamp rows on both ends: row i of A2 = padded row i-1, rows -1 and
        # 1028 duplicated (their values only reach non-output partitions)
        A2 = np.vstack([A[0:1], A, A[-1:]])  # [1030, 2052]
        P = (A2[:, 0:F].astype(np.uint16)
             | (A2[:, W // 2:W // 2 + F].astype(np.uint16) << 8)).view(np.int16)
        packed.append(P)
        raw.append(A2)
    in_maps = []
    for c in range(NCORES):
        b, qq = divmod(c, 4)
        A2 = raw[b]
        r0 = NBAND * BAND  # first strip out-row
        c0 = 512 * qq
        slo = A2[r0:r0 + SROWS + 6, c0:c0 + SF]
        shi = A2[r0:r0 + SROWS + 6, c0 + SF - 4:c0 + 2 * SF - 4]
        sp = (slo.astype(np.uint16) | (shi.astype(np.uint16) << 8)).view(np.int16)
        k0, k1 = 2 * qq, 2 * qq + 1
        im = {
            "s0": np.ascontiguousarray(packed[b][BAND * k0:BAND * k0 + 130, :]),
            "s1": np.ascontiguousarray(packed[b][BAND * k1:BAND * k1 + 130, :]),
            "ss": np.ascontiguousarray(sp),
            "wcat": wcat,
        }
        in_maps.append(im)
    return in_maps


def assemble(results):
    out = np.zeros((B, 1, H, W), np.int32)
    for c in range(NCORES):
        b, qq = divmod(c, 4)
        for j, k in enumerate((2 * qq, 2 * qq + 1)):
            v = results[c][f"y{j}"].reshape(BAND, F - 4, 2)
            rows = slice(BAND * k, BAND * (k + 1))
            out[b, 0, rows, 0:W // 2] = v[:, :, 0] != 0
            out[b, 0, rows, W // 2:W] = v[:, :, 1] != 0
        vs = results[c]["ys"].reshape(SROWS, SF - 4, 2)
        c0 = 512 * qq
        out[b, 0, NBAND * BAND:H, c0:c0 + 256] = vs[:, :, 0] != 0
        out[b, 0, NBAND * BAND:H, c0 + 256:c0 + 512] = vs[:, :, 1] != 0
    return out


def kernel(gtmasks):
    global LAST_EXEC_NS, LAST_RESULTS
    in_maps = make_in_maps(gtmasks)
    nc = build_nc()
    res = bass_utils.run_bass_kernel_spmd(
        nc, in_maps, core_ids=list(range(NCORES)), trace=PROFILE)
    LAST_EXEC_NS = res.exec_time_ns
    LAST_RESULTS = res
    return assemble(res.results)
